# revision 41
# baseline (speedup 1.0000x reference)
"""Trainium2 Bass kernel for nn_FDS (segment mean/var + EMA + conv smoothing).

Strategy (data-parallel over the row axis, 8 NeuronCores):
  - Each core gets a contiguous shard of 32768 rows of `features` (fp32) plus a
    [128, 256] relayout of its labels (one label per (partition, row-tile)).
  - On device: for each 128-row tile, build a [128, 50] one-hot of the labels on
    VectorE (is_equal against a [128, 50] bin-index constant), cast the feature
    tile to bf16 (VectorE), square it on ScalarE (fp32 in -> bf16 out), and use
    TensorE matmuls  onehot.T @ x  /  onehot.T @ x^2  accumulated in PSUM across
    all 256 tiles -> per-core per-bin partial sums/sumsq [50, 512] fp32.
  - Host: sum the 8 partials, bincount the labels for counts, then finalize
    mean/var, EMA against the running stats, and the 5-tap reflect-pad conv
    smoothing over the bin axis (all on tiny [50, 512] arrays).

The heavy memory traffic (512 MiB of fp32 features) is all on-device and the
kernel is memory-bound by design; PSUM accumulation is fp32 so only the bf16
rounding of x and x^2 touches precision (diluted ~sqrt(n) by the per-bin
averaging and a further 10x by the 0.9-momentum EMA).
"""

import numpy as np

import concourse.bacc as bacc
import concourse.bass as bass
import concourse.tile as tile
from concourse import mybir
from concourse.bass_utils import run_bass_kernel_spmd

N = 262144
D = 512
N_BINS = 50
N_CORES = 8
P = 128
ROWS_PER_CORE = N // N_CORES          # 32768
N_TILES = ROWS_PER_CORE // P          # 256 row-tiles per core
TILES_PER_CHUNK = 4                   # 512 rows = 1 MiB fp32 per DMA
KS = 5
HALF_KS = (KS - 1) // 2
MOMENTUM = 0.9

_NC_CACHE: dict = {}


def build_nc(n_tiles: int = N_TILES, tiles_per_chunk: int = TILES_PER_CHUNK) -> bass.Bass:
    """Build the per-core Bass program (same program on all 8 cores)."""
    assert n_tiles % tiles_per_chunk == 0
    n_chunks = n_tiles // tiles_per_chunk
    rows = n_tiles * P
    f32 = mybir.dt.float32
    bf16 = mybir.dt.bfloat16

    # Bacc (not raw Bass): its compile() pass splits excess per-instruction
    # semaphore waits onto EventSemaphore instructions, which TRN2's
    # fixed-size instruction structs require (most opcodes fit only 1 wait).
    nc = bacc.Bacc("TRN2", target_bir_lowering=False, debug=False)
    feat = nc.dram_tensor("feat", [rows, D], f32, kind="ExternalInput")
    # consts[:, :n_tiles] = per-(partition, tile) labels; consts[:, n_tiles:] =
    # bin indices 0..N_BINS-1 replicated tiles_per_chunk times, so one
    # is_equal op builds a whole chunk's worth of one-hot matrices. One
    # tensor so a single DMA covers every constant the one-hot op reads.
    consts = nc.dram_tensor(
        "consts", [P, n_tiles + tiles_per_chunk * N_BINS], f32,
        kind="ExternalInput",
    )
    # Rows 0:50 hold the even-tile partial, rows 64:114 the odd-tile partial
    # (PE column-packing puts odd tiles in array col-groups 2-3, whose PSUM
    # output lands at partition base 64). Host adds the halves.
    sum_out = nc.dram_tensor("sum_out", [P, D], f32, kind="ExternalOutput")
    sq_out = nc.dram_tensor("sq_out", [P, D], f32, kind="ExternalOutput")

    with tile.TileContext(nc) as tc:
        with (
            tc.tile_pool(name="const", bufs=1) as const_pool,
            # xf bufs == the 8 SW-DMA lanes: slot i always reloaded from the
            # same lane, so the cross-iteration WAW fence is implicit in lane
            # FIFO order and the DMACopy keeps a single sync wait (its ISA
            # struct has one wait slot).
            tc.tile_pool(name="xf", bufs=10) as xf_pool,
            tc.tile_pool(name="xb", bufs=8) as xb_pool,
            tc.tile_pool(name="xq", bufs=8) as xq_pool,
            tc.tile_pool(name="oh", bufs=1) as oh_pool,
            tc.tile_pool(name="res", bufs=1) as res_pool,
            tc.tile_pool(name="psum", bufs=1, space="PSUM") as psum_pool,
        ):
            consts_sb = const_pool.tile(
                [P, n_tiles + tiles_per_chunk * N_BINS], f32
            )
            nc.sync.dma_start(consts_sb[:], consts[:])
            lab_sb = consts_sb[:, :n_tiles]
            # [P, tiles_per_chunk, N_BINS] view of the replicated bin indices
            binidx_sb = consts_sb[:, n_tiles:].rearrange(
                "p (j b) -> p j b", b=N_BINS
            )

            # Four accumulation chains, one PSUM bank each: {sum, sumsq} x
            # {even tiles -> array col-groups 0-1 / PSUM partitions 0:50,
            #  odd tiles  -> array col-groups 2-3 / PSUM partitions 64:114}.
            # M=50 uses only 50 of the PE's 128 weight columns, so the even
            # and odd matmuls run CONCURRENTLY in different column groups —
            # halving effective PE time. Separate banks keep each chain's
            # start= bank-clear away from the others.
            acc_s_e = psum_pool.tile([P, D], f32)
            acc_s_o = psum_pool.tile([P, D], f32)
            acc_q_e = psum_pool.tile([P, D], f32)
            acc_q_o = psum_pool.tile([P, D], f32)

            # One persistent buffer holding every tile's one-hot matrix.
            # Unique [:, t, :] slices (never reused) keep the DVE
            # TensorScalar ops at <=1 semaphore wait — walrus rejects more
            # on that opcode — and avoid DVE<->PE slot-recycle stalls.
            oh_all = oh_pool.tile([P, n_tiles, N_BINS], bf16)

            for c in range(n_chunks):
                r0 = c * tiles_per_chunk * P
                chunk = feat[r0 : r0 + tiles_per_chunk * P, :].rearrange(
                    "(j p) d -> p j d", p=P
                )
                # Alternate loads between the SWDGE (gpsimd) and HWDGE
                # (sync) descriptor-generation paths: the SDMA data time is
                # shared either way, but each path's fixed per-DMA cost
                # (descriptor emission + completion receipt) overlaps the
                # other's instead of serializing into the chunk cadence.
                xf = xf_pool.tile([P, tiles_per_chunk, D], f32)
                if c % 2 == 0:
                    nc.gpsimd.dma_start(xf[:], chunk)
                else:
                    nc.sync.dma_start(xf[:], chunk)
                # Casts all on VectorE (ScalarE's activate-copy path is
                # ~2x slower); squares split 3:1 ScalarE:VectorE so neither
                # engine's per-chunk time approaches the DMA period.
                t0 = c * tiles_per_chunk
                nsc = tiles_per_chunk - 1
                xb = xb_pool.tile([P, tiles_per_chunk, D], bf16)
                nc.vector.tensor_copy(xb[:], xf[:])
                xq = xq_pool.tile([P, tiles_per_chunk, D], bf16)
                nc.scalar.activation(
                    xq[:, 0:nsc, :], xb[:, 0:nsc, :],
                    mybir.ActivationFunctionType.Square,
                )
                nc.vector.tensor_tensor(
                    xq[:, nsc, :], xb[:, nsc, :], xb[:, nsc, :],
                    op=mybir.AluOpType.mult,
                )
                # Whole chunk's one-hot matrices in a single DVE op: the
                # label column broadcasts (stride-0) across the bin axis
                # against the replicated bin-index block.
                nc.vector.tensor_tensor(
                    oh_all[:, t0 : t0 + tiles_per_chunk, :],
                    binidx_sb,
                    lab_sb[:, t0 : t0 + tiles_per_chunk].broadcast_to(
                        [P, tiles_per_chunk, N_BINS]
                    ),
                    op=mybir.AluOpType.is_equal,
                )
                for j in range(0, tiles_per_chunk, 2):
                    t = c * tiles_per_chunk + j
                    oh_e = oh_all[:, t, :]
                    oh_o = oh_all[:, t + 1, :]
                    first, last = t == 0, t == n_tiles - 2
                    # Interleave even/odd so consecutive matmuls target
                    # different column groups and overlap in the array.
                    nc.tensor.matmul(
                        acc_s_e[0:N_BINS, :], oh_e, xb[:, j, :],
                        start=first, stop=last,
                    )
                    nc.tensor.matmul(
                        acc_s_o[64 : 64 + N_BINS, :], oh_o, xb[:, j + 1, :],
                        start=first, stop=last,
                    )
                    nc.tensor.matmul(
                        acc_q_e[0:N_BINS, :], oh_e, xq[:, j, :],
                        start=first, stop=last,
                    )
                    nc.tensor.matmul(
                        acc_q_o[64 : 64 + N_BINS, :], oh_o, xq[:, j + 1, :],
                        start=first, stop=last,
                    )

            res_s = res_pool.tile([P, D], f32)
            res_q = res_pool.tile([P, D], f32)
            nc.gpsimd.memset(res_s[:], 0.0)
            nc.gpsimd.memset(res_q[:], 0.0)
            nc.vector.tensor_copy(res_s[0:N_BINS, :], acc_s_e[0:N_BINS, :])
            nc.vector.tensor_copy(
                res_s[64 : 64 + N_BINS, :], acc_s_o[64 : 64 + N_BINS, :]
            )
            nc.gpsimd.dma_start(sum_out[:], res_s[:])
            nc.scalar.copy(res_q[0:N_BINS, :], acc_q_e[0:N_BINS, :])
            nc.scalar.copy(
                res_q[64 : 64 + N_BINS, :], acc_q_o[64 : 64 + N_BINS, :]
            )
            nc.gpsimd.dma_start(sq_out[:], res_q[:])

    nc.finalize()
    return nc


def make_in_maps(features: np.ndarray, labels_f32: np.ndarray) -> list[dict]:
    """Shard host inputs into the 8 per-core input maps."""
    binidx = np.broadcast_to(
        np.arange(N_BINS, dtype=np.float32), (P, TILES_PER_CHUNK, N_BINS)
    ).reshape(P, TILES_PER_CHUNK * N_BINS)
    in_maps = []
    for c in range(N_CORES):
        lo = c * ROWS_PER_CORE
        shard = features[lo : lo + ROWS_PER_CORE]
        lab_pt = labels_f32[lo : lo + ROWS_PER_CORE].reshape(N_TILES, P).T
        consts = np.ascontiguousarray(np.concatenate([lab_pt, binidx], axis=1))
        in_maps.append({"feat": shard, "consts": consts})
    return in_maps


def _smooth(x: np.ndarray, w: np.ndarray) -> np.ndarray:
    # torch-style 'reflect' pad (edge-excluding) along bins, then cross-correlate.
    top = x[HALF_KS:0:-1]
    bot = x[-2 : -2 - HALF_KS : -1]
    xp = np.concatenate([top, x, bot], axis=0)
    out = np.zeros_like(x)
    b = x.shape[0]
    for k in range(KS):
        out += w[k] * xp[k : k + b]
    return out


def finalize(
    sums: np.ndarray,
    sumsq: np.ndarray,
    counts: np.ndarray,
    running_mean: np.ndarray,
    running_var: np.ndarray,
    num_samples_tracked: np.ndarray,
    kernel_window: np.ndarray,
):
    """Replicates reference() from the per-bin partials (float64 internally)."""
    counts = counts.astype(np.float64)
    safe_n = np.maximum(counts, 1.0)[:, None]
    mean = sums / safe_n
    var = (sumsq - safe_n * mean * mean) / np.maximum(counts - 1.0, 1.0)[:, None]

    present = (counts > 0.0)[:, None]
    f = MOMENTUM
    rm = running_mean.astype(np.float64)
    rv = running_var.astype(np.float64)
    new_mean = np.where(present, (1.0 - f) * mean + f * rm, rm)
    new_var = np.where(present, (1.0 - f) * var + f * rv, rv)
    new_num = num_samples_tracked.astype(np.float64) + counts

    w = kernel_window.astype(np.float64)
    smoothed_mean = _smooth(new_mean, w)
    smoothed_var = _smooth(new_var, w)
    return (
        new_mean.astype(np.float32),
        new_var.astype(np.float32),
        new_num.astype(np.float32),
        smoothed_mean.astype(np.float32),
        smoothed_var.astype(np.float32),
    )


def kernel(features, labels, running_mean, running_var, num_samples_tracked, kernel_window):
    features = np.ascontiguousarray(np.asarray(features), dtype=np.float32)
    labels_i = np.asarray(labels).astype(np.int64)
    labels_f32 = labels_i.astype(np.float32)
    running_mean = np.asarray(running_mean, dtype=np.float32)
    running_var = np.asarray(running_var, dtype=np.float32)
    num_samples_tracked = np.asarray(num_samples_tracked, dtype=np.float32)
    kernel_window = np.asarray(kernel_window, dtype=np.float32)

    if "nc" not in _NC_CACHE:
        _NC_CACHE["nc"] = build_nc()
    nc = _NC_CACHE["nc"]

    in_maps = make_in_maps(features, labels_f32)
    res = run_bass_kernel_spmd(nc, in_maps, list(range(N_CORES))).results

    sums = np.sum(
        [r["sum_out"][0:N_BINS] + r["sum_out"][64 : 64 + N_BINS] for r in res],
        axis=0, dtype=np.float64,
    )
    sumsq = np.sum(
        [r["sq_out"][0:N_BINS] + r["sq_out"][64 : 64 + N_BINS] for r in res],
        axis=0, dtype=np.float64,
    )
    counts = np.bincount(labels_i, minlength=N_BINS).astype(np.float64)

    return finalize(
        sums, sumsq, counts,
        running_mean, running_var, num_samples_tracked, kernel_window,
    )


# revision 43
# speedup vs baseline: 1.0193x; 1.0193x over previous
"""Trainium2 Bass kernel for nn_FDS (segment mean/var + EMA + conv smoothing).

Strategy (data-parallel over the row axis, 8 NeuronCores):
  - Each core gets a contiguous shard of 32768 rows of `features` (fp32) plus a
    [128, 256] relayout of its labels (one label per (partition, row-tile)).
  - On device: for each 128-row tile, build a [128, 50] one-hot of the labels on
    VectorE (is_equal against a [128, 50] bin-index constant), cast the feature
    tile to bf16 (VectorE), square it on ScalarE (fp32 in -> bf16 out), and use
    TensorE matmuls  onehot.T @ x  /  onehot.T @ x^2  accumulated in PSUM across
    all 256 tiles -> per-core per-bin partial sums/sumsq [50, 512] fp32.
  - Host: sum the 8 partials, bincount the labels for counts, then finalize
    mean/var, EMA against the running stats, and the 5-tap reflect-pad conv
    smoothing over the bin axis (all on tiny [50, 512] arrays).

The heavy memory traffic (512 MiB of fp32 features) is all on-device and the
kernel is memory-bound by design; PSUM accumulation is fp32 so only the bf16
rounding of x and x^2 touches precision (diluted ~sqrt(n) by the per-bin
averaging and a further 10x by the 0.9-momentum EMA).
"""

import numpy as np

import concourse.bacc as bacc
import concourse.bass as bass
import concourse.tile as tile
from concourse import mybir
from concourse.bass_utils import run_bass_kernel_spmd

N = 262144
D = 512
N_BINS = 50
N_CORES = 8
P = 128
ROWS_PER_CORE = N // N_CORES          # 32768
N_TILES = ROWS_PER_CORE // P          # 256 row-tiles per core
TILES_PER_CHUNK = 4                   # 512 rows = 1 MiB fp32 per DMA
KS = 5
HALF_KS = (KS - 1) // 2
MOMENTUM = 0.9

_NC_CACHE: dict = {}


def build_nc(n_tiles: int = N_TILES, tiles_per_chunk: int = TILES_PER_CHUNK) -> bass.Bass:
    """Build the per-core Bass program (same program on all 8 cores)."""
    assert n_tiles % tiles_per_chunk == 0
    n_chunks = n_tiles // tiles_per_chunk
    rows = n_tiles * P
    f32 = mybir.dt.float32
    bf16 = mybir.dt.bfloat16

    # Bacc (not raw Bass): its compile() pass splits excess per-instruction
    # semaphore waits onto EventSemaphore instructions, which TRN2's
    # fixed-size instruction structs require (most opcodes fit only 1 wait).
    nc = bacc.Bacc("TRN2", target_bir_lowering=False, debug=False)
    feat = nc.dram_tensor("feat", [rows, D], f32, kind="ExternalInput")
    # consts[:, :n_tiles] = per-(partition, tile) labels; consts[:, n_tiles:] =
    # bin indices 0..N_BINS-1 replicated tiles_per_chunk times, so one
    # is_equal op builds a whole chunk's worth of one-hot matrices. One
    # tensor so a single DMA covers every constant the one-hot op reads.
    consts = nc.dram_tensor(
        "consts", [P, n_tiles + tiles_per_chunk * N_BINS], f32,
        kind="ExternalInput",
    )
    # Rows 0:50 hold the even-tile partial, rows 64:114 the odd-tile partial
    # (PE column-packing puts odd tiles in array col-groups 2-3, whose PSUM
    # output lands at partition base 64). Host adds the halves.
    sum_out = nc.dram_tensor("sum_out", [P, D], f32, kind="ExternalOutput")
    sq_out = nc.dram_tensor("sq_out", [P, D], f32, kind="ExternalOutput")

    with tile.TileContext(nc) as tc:
        with (
            tc.tile_pool(name="const", bufs=1) as const_pool,
            # xf bufs == the 8 SW-DMA lanes: slot i always reloaded from the
            # same lane, so the cross-iteration WAW fence is implicit in lane
            # FIFO order and the DMACopy keeps a single sync wait (its ISA
            # struct has one wait slot).
            tc.tile_pool(name="xf", bufs=12) as xf_pool,
            tc.tile_pool(name="xb", bufs=5) as xb_pool,
            tc.tile_pool(name="xq", bufs=5) as xq_pool,
            tc.tile_pool(name="oh", bufs=1) as oh_pool,
            tc.tile_pool(name="res", bufs=1) as res_pool,
            tc.tile_pool(name="psum", bufs=1, space="PSUM") as psum_pool,
        ):
            consts_sb = const_pool.tile(
                [P, n_tiles + tiles_per_chunk * N_BINS], f32
            )
            nc.sync.dma_start(consts_sb[:], consts[:])
            lab_sb = consts_sb[:, :n_tiles]
            # [P, tiles_per_chunk, N_BINS] view of the replicated bin indices
            binidx_sb = consts_sb[:, n_tiles:].rearrange(
                "p (j b) -> p j b", b=N_BINS
            )

            # Four accumulation chains, one PSUM bank each: {sum, sumsq} x
            # {even tiles -> array col-groups 0-1 / PSUM partitions 0:50,
            #  odd tiles  -> array col-groups 2-3 / PSUM partitions 64:114}.
            # M=50 uses only 50 of the PE's 128 weight columns, so the even
            # and odd matmuls run CONCURRENTLY in different column groups —
            # halving effective PE time. Separate banks keep each chain's
            # start= bank-clear away from the others.
            acc_s_e = psum_pool.tile([P, D], f32)
            acc_s_o = psum_pool.tile([P, D], f32)
            acc_q_e = psum_pool.tile([P, D], f32)
            acc_q_o = psum_pool.tile([P, D], f32)

            # One persistent buffer holding every tile's one-hot matrix.
            # Unique [:, t, :] slices (never reused) keep the DVE
            # TensorScalar ops at <=1 semaphore wait — walrus rejects more
            # on that opcode — and avoid DVE<->PE slot-recycle stalls.
            oh_all = oh_pool.tile([P, n_tiles, N_BINS], bf16)

            for c in range(n_chunks):
                r0 = c * tiles_per_chunk * P
                chunk = feat[r0 : r0 + tiles_per_chunk * P, :].rearrange(
                    "(j p) d -> p j d", p=P
                )
                # SWDGE (gpsimd) DMA — Bacc splits surplus sync waits onto
                # EventSemaphore instructions. (Alternating loads with the
                # HWDGE path measured slower; keep a single DGE path.)
                xf = xf_pool.tile([P, tiles_per_chunk, D], f32)
                nc.gpsimd.dma_start(xf[:], chunk)
                # Casts all on VectorE (ScalarE's activate-copy path is
                # ~2x slower); squares split 3:1 ScalarE:VectorE so neither
                # engine's per-chunk time approaches the DMA period.
                t0 = c * tiles_per_chunk
                nsc = tiles_per_chunk - 1
                xb = xb_pool.tile([P, tiles_per_chunk, D], bf16)
                nc.vector.tensor_copy(xb[:], xf[:])
                xq = xq_pool.tile([P, tiles_per_chunk, D], bf16)
                nc.scalar.activation(
                    xq[:, 0:nsc, :], xb[:, 0:nsc, :],
                    mybir.ActivationFunctionType.Square,
                )
                nc.vector.tensor_tensor(
                    xq[:, nsc, :], xb[:, nsc, :], xb[:, nsc, :],
                    op=mybir.AluOpType.mult,
                )
                # Whole chunk's one-hot matrices in a single DVE op: the
                # label column broadcasts (stride-0) across the bin axis
                # against the replicated bin-index block.
                nc.vector.tensor_tensor(
                    oh_all[:, t0 : t0 + tiles_per_chunk, :],
                    binidx_sb,
                    lab_sb[:, t0 : t0 + tiles_per_chunk].broadcast_to(
                        [P, tiles_per_chunk, N_BINS]
                    ),
                    op=mybir.AluOpType.is_equal,
                )
                for j in range(0, tiles_per_chunk, 2):
                    t = c * tiles_per_chunk + j
                    oh_e = oh_all[:, t, :]
                    oh_o = oh_all[:, t + 1, :]
                    first, last = t == 0, t == n_tiles - 2
                    # Interleave even/odd so consecutive matmuls target
                    # different column groups and overlap in the array.
                    nc.tensor.matmul(
                        acc_s_e[0:N_BINS, :], oh_e, xb[:, j, :],
                        start=first, stop=last,
                    )
                    nc.tensor.matmul(
                        acc_s_o[64 : 64 + N_BINS, :], oh_o, xb[:, j + 1, :],
                        start=first, stop=last,
                    )
                    nc.tensor.matmul(
                        acc_q_e[0:N_BINS, :], oh_e, xq[:, j, :],
                        start=first, stop=last,
                    )
                    nc.tensor.matmul(
                        acc_q_o[64 : 64 + N_BINS, :], oh_o, xq[:, j + 1, :],
                        start=first, stop=last,
                    )

            res_s = res_pool.tile([P, D], f32)
            res_q = res_pool.tile([P, D], f32)
            nc.gpsimd.memset(res_s[:], 0.0)
            nc.gpsimd.memset(res_q[:], 0.0)
            nc.vector.tensor_copy(res_s[0:N_BINS, :], acc_s_e[0:N_BINS, :])
            nc.vector.tensor_copy(
                res_s[64 : 64 + N_BINS, :], acc_s_o[64 : 64 + N_BINS, :]
            )
            nc.gpsimd.dma_start(sum_out[:], res_s[:])
            nc.scalar.copy(res_q[0:N_BINS, :], acc_q_e[0:N_BINS, :])
            nc.scalar.copy(
                res_q[64 : 64 + N_BINS, :], acc_q_o[64 : 64 + N_BINS, :]
            )
            nc.gpsimd.dma_start(sq_out[:], res_q[:])

    nc.finalize()
    return nc


def make_in_maps(features: np.ndarray, labels_f32: np.ndarray) -> list[dict]:
    """Shard host inputs into the 8 per-core input maps."""
    binidx = np.broadcast_to(
        np.arange(N_BINS, dtype=np.float32), (P, TILES_PER_CHUNK, N_BINS)
    ).reshape(P, TILES_PER_CHUNK * N_BINS)
    in_maps = []
    for c in range(N_CORES):
        lo = c * ROWS_PER_CORE
        shard = features[lo : lo + ROWS_PER_CORE]
        lab_pt = labels_f32[lo : lo + ROWS_PER_CORE].reshape(N_TILES, P).T
        consts = np.ascontiguousarray(np.concatenate([lab_pt, binidx], axis=1))
        in_maps.append({"feat": shard, "consts": consts})
    return in_maps


def _smooth(x: np.ndarray, w: np.ndarray) -> np.ndarray:
    # torch-style 'reflect' pad (edge-excluding) along bins, then cross-correlate.
    top = x[HALF_KS:0:-1]
    bot = x[-2 : -2 - HALF_KS : -1]
    xp = np.concatenate([top, x, bot], axis=0)
    out = np.zeros_like(x)
    b = x.shape[0]
    for k in range(KS):
        out += w[k] * xp[k : k + b]
    return out


def finalize(
    sums: np.ndarray,
    sumsq: np.ndarray,
    counts: np.ndarray,
    running_mean: np.ndarray,
    running_var: np.ndarray,
    num_samples_tracked: np.ndarray,
    kernel_window: np.ndarray,
):
    """Replicates reference() from the per-bin partials (float64 internally)."""
    counts = counts.astype(np.float64)
    safe_n = np.maximum(counts, 1.0)[:, None]
    mean = sums / safe_n
    var = (sumsq - safe_n * mean * mean) / np.maximum(counts - 1.0, 1.0)[:, None]

    present = (counts > 0.0)[:, None]
    f = MOMENTUM
    rm = running_mean.astype(np.float64)
    rv = running_var.astype(np.float64)
    new_mean = np.where(present, (1.0 - f) * mean + f * rm, rm)
    new_var = np.where(present, (1.0 - f) * var + f * rv, rv)
    new_num = num_samples_tracked.astype(np.float64) + counts

    w = kernel_window.astype(np.float64)
    smoothed_mean = _smooth(new_mean, w)
    smoothed_var = _smooth(new_var, w)
    return (
        new_mean.astype(np.float32),
        new_var.astype(np.float32),
        new_num.astype(np.float32),
        smoothed_mean.astype(np.float32),
        smoothed_var.astype(np.float32),
    )


def kernel(features, labels, running_mean, running_var, num_samples_tracked, kernel_window):
    features = np.ascontiguousarray(np.asarray(features), dtype=np.float32)
    labels_i = np.asarray(labels).astype(np.int64)
    labels_f32 = labels_i.astype(np.float32)
    running_mean = np.asarray(running_mean, dtype=np.float32)
    running_var = np.asarray(running_var, dtype=np.float32)
    num_samples_tracked = np.asarray(num_samples_tracked, dtype=np.float32)
    kernel_window = np.asarray(kernel_window, dtype=np.float32)

    if "nc" not in _NC_CACHE:
        _NC_CACHE["nc"] = build_nc()
    nc = _NC_CACHE["nc"]

    in_maps = make_in_maps(features, labels_f32)
    res = run_bass_kernel_spmd(nc, in_maps, list(range(N_CORES))).results

    sums = np.sum(
        [r["sum_out"][0:N_BINS] + r["sum_out"][64 : 64 + N_BINS] for r in res],
        axis=0, dtype=np.float64,
    )
    sumsq = np.sum(
        [r["sq_out"][0:N_BINS] + r["sq_out"][64 : 64 + N_BINS] for r in res],
        axis=0, dtype=np.float64,
    )
    counts = np.bincount(labels_i, minlength=N_BINS).astype(np.float64)

    return finalize(
        sums, sumsq, counts,
        running_mean, running_var, num_samples_tracked, kernel_window,
    )


# revision 50
# speedup vs baseline: 1.1009x; 1.0800x over previous
"""Trainium2 Bass kernel for nn_FDS (segment mean/var + EMA + conv smoothing).

Strategy (data-parallel over the row axis, 8 NeuronCores):
  - Each core gets a contiguous shard of 32768 rows of `features` (fp32) plus a
    [128, 256] relayout of its labels (one label per (partition, row-tile)).
  - On device: for each 128-row tile, build a [128, 50] one-hot of the labels on
    VectorE (is_equal against a [128, 50] bin-index constant), cast the feature
    tile to bf16 (VectorE), square it on ScalarE (fp32 in -> bf16 out), and use
    TensorE matmuls  onehot.T @ x  /  onehot.T @ x^2  accumulated in PSUM across
    all 256 tiles -> per-core per-bin partial sums/sumsq [50, 512] fp32.
  - Host: sum the 8 partials, bincount the labels for counts, then finalize
    mean/var, EMA against the running stats, and the 5-tap reflect-pad conv
    smoothing over the bin axis (all on tiny [50, 512] arrays).

The heavy memory traffic (512 MiB of fp32 features) is all on-device and the
kernel is memory-bound by design; PSUM accumulation is fp32 so only the bf16
rounding of x and x^2 touches precision (diluted ~sqrt(n) by the per-bin
averaging and a further 10x by the 0.9-momentum EMA).
"""

import numpy as np

import concourse.bacc as bacc
import concourse.bass as bass
import concourse.tile as tile
from concourse import mybir
from concourse.bass_utils import run_bass_kernel_spmd

N = 262144
D = 512
N_BINS = 50
N_CORES = 8
P = 128
ROWS_PER_CORE = N // N_CORES          # 32768
N_TILES = ROWS_PER_CORE // P          # 256 row-tiles per core
TILES_PER_CHUNK = 8                   # 1024 rows = 2 MiB fp32 per DMA
BATCH = 4                             # compute granularity: 4-tile halves
KS = 5
HALF_KS = (KS - 1) // 2
MOMENTUM = 0.9

_NC_CACHE: dict = {}


def build_nc(
    n_tiles: int = N_TILES,
    tiles_per_chunk: int = TILES_PER_CHUNK,
    batch: int = BATCH,
) -> bass.Bass:
    """Build the per-core Bass program (same program on all 8 cores)."""
    assert n_tiles % tiles_per_chunk == 0
    assert tiles_per_chunk % batch == 0
    n_chunks = n_tiles // tiles_per_chunk
    rows = n_tiles * P
    f32 = mybir.dt.float32
    bf16 = mybir.dt.bfloat16

    # Bacc (not raw Bass): its compile() pass splits excess per-instruction
    # semaphore waits onto EventSemaphore instructions, which TRN2's
    # fixed-size instruction structs require (most opcodes fit only 1 wait).
    nc = bacc.Bacc("TRN2", target_bir_lowering=False, debug=False)
    feat = nc.dram_tensor("feat", [rows, D], f32, kind="ExternalInput")
    # consts[:, :n_tiles] = per-(partition, tile) labels; consts[:, n_tiles:] =
    # bin indices 0..N_BINS-1 replicated `batch` times, so one is_equal op
    # builds a compute-batch's worth of one-hot matrices. One tensor so a
    # single DMA covers every constant the one-hot op reads.
    consts = nc.dram_tensor(
        "consts", [P, n_tiles + batch * N_BINS], f32, kind="ExternalInput"
    )
    # Rows 0:50 hold the even-tile partial, rows 64:114 the odd-tile partial
    # (PE column-packing puts odd tiles in array col-groups 2-3, whose PSUM
    # output lands at partition base 64). Host adds the halves.
    sum_out = nc.dram_tensor("sum_out", [P, D], f32, kind="ExternalOutput")
    sq_out = nc.dram_tensor("sq_out", [P, D], f32, kind="ExternalOutput")

    with tile.TileContext(nc) as tc:
        with (
            tc.tile_pool(name="const", bufs=1) as const_pool,
            # xf bufs == the 8 SW-DMA lanes: slot i always reloaded from the
            # same lane, so the cross-iteration WAW fence is implicit in lane
            # FIFO order and the DMACopy keeps a single sync wait (its ISA
            # struct has one wait slot).
            tc.tile_pool(name="xf", bufs=6) as xf_pool,
            tc.tile_pool(name="xb", bufs=5) as xb_pool,
            tc.tile_pool(name="xq", bufs=5) as xq_pool,
            tc.tile_pool(name="oh", bufs=1) as oh_pool,
            tc.tile_pool(name="res", bufs=1) as res_pool,
            tc.tile_pool(name="psum", bufs=1, space="PSUM") as psum_pool,
        ):
            consts_sb = const_pool.tile([P, n_tiles + batch * N_BINS], f32)
            nc.sync.dma_start(consts_sb[:], consts[:])
            lab_sb = consts_sb[:, :n_tiles]
            # [P, batch, N_BINS] view of the replicated bin indices
            binidx_sb = consts_sb[:, n_tiles:].rearrange(
                "p (j b) -> p j b", b=N_BINS
            )

            # Four accumulation chains, one PSUM bank each: {sum, sumsq} x
            # {even tiles -> array col-groups 0-1 / PSUM partitions 0:50,
            #  odd tiles  -> array col-groups 2-3 / PSUM partitions 64:114}.
            # M=50 uses only 50 of the PE's 128 weight columns, so the even
            # and odd matmuls run CONCURRENTLY in different column groups —
            # halving effective PE time. Separate banks keep each chain's
            # start= bank-clear away from the others.
            acc_s_e = psum_pool.tile([P, D], f32)
            acc_s_o = psum_pool.tile([P, D], f32)
            acc_q_e = psum_pool.tile([P, D], f32)
            acc_q_o = psum_pool.tile([P, D], f32)

            # One persistent buffer holding every tile's one-hot matrix.
            # Unique [:, t, :] slices (never reused) keep the DVE
            # TensorScalar ops at <=1 semaphore wait — walrus rejects more
            # on that opcode — and avoid DVE<->PE slot-recycle stalls.
            oh_all = oh_pool.tile([P, n_tiles, N_BINS], bf16)

            for c in range(n_chunks):
                r0 = c * tiles_per_chunk * P
                chunk = feat[r0 : r0 + tiles_per_chunk * P, :].rearrange(
                    "(j p) d -> p j d", p=P
                )
                # One 2 MiB SWDGE (gpsimd) DMA per chunk halves the per-DMA
                # fixed cost (descriptor emission + completion receipt) that
                # was stretching the chunk cadence past the pure data time.
                xf = xf_pool.tile([P, tiles_per_chunk, D], f32)
                nc.gpsimd.dma_start(xf[:], chunk)
                # Compute stays at `batch`-tile granularity so pipeline
                # latency (and the end-of-stream drain) stays short.
                for h in range(tiles_per_chunk // batch):
                    hs = h * batch
                    t0 = c * tiles_per_chunk + hs
                    xfh = xf[:, hs : hs + batch, :]
                    # Casts all on VectorE (ScalarE's activate-copy path is
                    # ~2x slower); squares split 3:1 ScalarE:VectorE so
                    # neither engine approaches the DMA period.
                    nsc = batch - 1
                    xb = xb_pool.tile([P, batch, D], bf16)
                    nc.vector.tensor_copy(xb[:], xfh)
                    xq = xq_pool.tile([P, batch, D], bf16)
                    nc.scalar.activation(
                        xq[:, 0:nsc, :], xb[:, 0:nsc, :],
                        mybir.ActivationFunctionType.Square,
                    )
                    nc.vector.tensor_tensor(
                        xq[:, nsc, :], xb[:, nsc, :], xb[:, nsc, :],
                        op=mybir.AluOpType.mult,
                    )
                    # Whole batch's one-hot matrices in a single DVE op: the
                    # label column broadcasts (stride-0) across the bin axis
                    # against the replicated bin-index block.
                    nc.vector.tensor_tensor(
                        oh_all[:, t0 : t0 + batch, :],
                        binidx_sb,
                        lab_sb[:, t0 : t0 + batch].broadcast_to(
                            [P, batch, N_BINS]
                        ),
                        op=mybir.AluOpType.is_equal,
                    )
                    for j in range(0, batch, 2):
                        t = t0 + j
                        oh_e = oh_all[:, t, :]
                        oh_o = oh_all[:, t + 1, :]
                        first, last = t == 0, t == n_tiles - 2
                        # Interleave even/odd so consecutive matmuls target
                        # different column groups and overlap in the array.
                        nc.tensor.matmul(
                            acc_s_e[0:N_BINS, :], oh_e, xb[:, j, :],
                            start=first, stop=last,
                        )
                        nc.tensor.matmul(
                            acc_s_o[64 : 64 + N_BINS, :], oh_o,
                            xb[:, j + 1, :],
                            start=first, stop=last,
                        )
                        nc.tensor.matmul(
                            acc_q_e[0:N_BINS, :], oh_e, xq[:, j, :],
                            start=first, stop=last,
                        )
                        nc.tensor.matmul(
                            acc_q_o[64 : 64 + N_BINS, :], oh_o,
                            xq[:, j + 1, :],
                            start=first, stop=last,
                        )

            res_s = res_pool.tile([P, D], f32)
            res_q = res_pool.tile([P, D], f32)
            nc.gpsimd.memset(res_s[:], 0.0)
            nc.gpsimd.memset(res_q[:], 0.0)
            nc.vector.tensor_copy(res_s[0:N_BINS, :], acc_s_e[0:N_BINS, :])
            nc.vector.tensor_copy(
                res_s[64 : 64 + N_BINS, :], acc_s_o[64 : 64 + N_BINS, :]
            )
            nc.gpsimd.dma_start(sum_out[:], res_s[:])
            nc.scalar.copy(res_q[0:N_BINS, :], acc_q_e[0:N_BINS, :])
            nc.scalar.copy(
                res_q[64 : 64 + N_BINS, :], acc_q_o[64 : 64 + N_BINS, :]
            )
            nc.gpsimd.dma_start(sq_out[:], res_q[:])

    nc.finalize()
    return nc


def make_in_maps(features: np.ndarray, labels_f32: np.ndarray) -> list[dict]:
    """Shard host inputs into the 8 per-core input maps."""
    binidx = np.broadcast_to(
        np.arange(N_BINS, dtype=np.float32), (P, BATCH, N_BINS)
    ).reshape(P, BATCH * N_BINS)
    in_maps = []
    for c in range(N_CORES):
        lo = c * ROWS_PER_CORE
        shard = features[lo : lo + ROWS_PER_CORE]
        lab_pt = labels_f32[lo : lo + ROWS_PER_CORE].reshape(N_TILES, P).T
        consts = np.ascontiguousarray(np.concatenate([lab_pt, binidx], axis=1))
        in_maps.append({"feat": shard, "consts": consts})
    return in_maps


def _smooth(x: np.ndarray, w: np.ndarray) -> np.ndarray:
    # torch-style 'reflect' pad (edge-excluding) along bins, then cross-correlate.
    top = x[HALF_KS:0:-1]
    bot = x[-2 : -2 - HALF_KS : -1]
    xp = np.concatenate([top, x, bot], axis=0)
    out = np.zeros_like(x)
    b = x.shape[0]
    for k in range(KS):
        out += w[k] * xp[k : k + b]
    return out


def finalize(
    sums: np.ndarray,
    sumsq: np.ndarray,
    counts: np.ndarray,
    running_mean: np.ndarray,
    running_var: np.ndarray,
    num_samples_tracked: np.ndarray,
    kernel_window: np.ndarray,
):
    """Replicates reference() from the per-bin partials (float64 internally)."""
    counts = counts.astype(np.float64)
    safe_n = np.maximum(counts, 1.0)[:, None]
    mean = sums / safe_n
    var = (sumsq - safe_n * mean * mean) / np.maximum(counts - 1.0, 1.0)[:, None]

    present = (counts > 0.0)[:, None]
    f = MOMENTUM
    rm = running_mean.astype(np.float64)
    rv = running_var.astype(np.float64)
    new_mean = np.where(present, (1.0 - f) * mean + f * rm, rm)
    new_var = np.where(present, (1.0 - f) * var + f * rv, rv)
    new_num = num_samples_tracked.astype(np.float64) + counts

    w = kernel_window.astype(np.float64)
    smoothed_mean = _smooth(new_mean, w)
    smoothed_var = _smooth(new_var, w)
    return (
        new_mean.astype(np.float32),
        new_var.astype(np.float32),
        new_num.astype(np.float32),
        smoothed_mean.astype(np.float32),
        smoothed_var.astype(np.float32),
    )


def kernel(features, labels, running_mean, running_var, num_samples_tracked, kernel_window):
    features = np.ascontiguousarray(np.asarray(features), dtype=np.float32)
    labels_i = np.asarray(labels).astype(np.int64)
    labels_f32 = labels_i.astype(np.float32)
    running_mean = np.asarray(running_mean, dtype=np.float32)
    running_var = np.asarray(running_var, dtype=np.float32)
    num_samples_tracked = np.asarray(num_samples_tracked, dtype=np.float32)
    kernel_window = np.asarray(kernel_window, dtype=np.float32)

    if "nc" not in _NC_CACHE:
        _NC_CACHE["nc"] = build_nc()
    nc = _NC_CACHE["nc"]

    in_maps = make_in_maps(features, labels_f32)
    res = run_bass_kernel_spmd(nc, in_maps, list(range(N_CORES))).results

    sums = np.sum(
        [r["sum_out"][0:N_BINS] + r["sum_out"][64 : 64 + N_BINS] for r in res],
        axis=0, dtype=np.float64,
    )
    sumsq = np.sum(
        [r["sq_out"][0:N_BINS] + r["sq_out"][64 : 64 + N_BINS] for r in res],
        axis=0, dtype=np.float64,
    )
    counts = np.bincount(labels_i, minlength=N_BINS).astype(np.float64)

    return finalize(
        sums, sumsq, counts,
        running_mean, running_var, num_samples_tracked, kernel_window,
    )


# revision 52
# speedup vs baseline: 1.1155x; 1.0133x over previous
"""Trainium2 Bass kernel for nn_FDS (segment mean/var + EMA + conv smoothing).

Strategy (data-parallel over the row axis, 8 NeuronCores):
  - Each core gets a contiguous shard of 32768 rows of `features` (fp32) plus a
    [128, 256] relayout of its labels (one label per (partition, row-tile)).
  - On device: for each 128-row tile, build a [128, 50] one-hot of the labels on
    VectorE (is_equal against a [128, 50] bin-index constant), cast the feature
    tile to bf16 (VectorE), square it on ScalarE (fp32 in -> bf16 out), and use
    TensorE matmuls  onehot.T @ x  /  onehot.T @ x^2  accumulated in PSUM across
    all 256 tiles -> per-core per-bin partial sums/sumsq [50, 512] fp32.
  - Host: sum the 8 partials, bincount the labels for counts, then finalize
    mean/var, EMA against the running stats, and the 5-tap reflect-pad conv
    smoothing over the bin axis (all on tiny [50, 512] arrays).

The heavy memory traffic (512 MiB of fp32 features) is all on-device and the
kernel is memory-bound by design; PSUM accumulation is fp32 so only the bf16
rounding of x and x^2 touches precision (diluted ~sqrt(n) by the per-bin
averaging and a further 10x by the 0.9-momentum EMA).
"""

import numpy as np

import concourse.bacc as bacc
import concourse.bass as bass
import concourse.tile as tile
from concourse import mybir
from concourse.bass_utils import run_bass_kernel_spmd

N = 262144
D = 512
N_BINS = 50
N_CORES = 8
P = 128
ROWS_PER_CORE = N // N_CORES          # 32768
N_TILES = ROWS_PER_CORE // P          # 256 row-tiles per core
TILES_PER_CHUNK = 8                   # 1024 rows = 2 MiB fp32 per DMA
BATCH = 4                             # compute granularity: 4-tile halves
KS = 5
HALF_KS = (KS - 1) // 2
MOMENTUM = 0.9

_NC_CACHE: dict = {}


def build_nc(
    n_tiles: int = N_TILES,
    tiles_per_chunk: int = TILES_PER_CHUNK,
    batch: int = BATCH,
) -> bass.Bass:
    """Build the per-core Bass program (same program on all 8 cores)."""
    assert n_tiles % tiles_per_chunk == 0
    assert tiles_per_chunk % batch == 0
    n_chunks = n_tiles // tiles_per_chunk
    rows = n_tiles * P
    f32 = mybir.dt.float32
    bf16 = mybir.dt.bfloat16

    # Bacc (not raw Bass): its compile() pass splits excess per-instruction
    # semaphore waits onto EventSemaphore instructions, which TRN2's
    # fixed-size instruction structs require (most opcodes fit only 1 wait).
    nc = bacc.Bacc("TRN2", target_bir_lowering=False, debug=False)
    feat = nc.dram_tensor("feat", [rows, D], f32, kind="ExternalInput")
    # consts[:, :n_tiles] = per-(partition, tile) labels; consts[:, n_tiles:] =
    # bin indices 0..N_BINS-1 replicated `batch` times, so one is_equal op
    # builds a compute-batch's worth of one-hot matrices. One tensor so a
    # single DMA covers every constant the one-hot op reads.
    consts = nc.dram_tensor(
        "consts", [P, n_tiles + batch * N_BINS], f32, kind="ExternalInput"
    )
    # Rows 0:50 hold the even-tile partial, rows 64:114 the odd-tile partial
    # (PE column-packing puts odd tiles in array col-groups 2-3, whose PSUM
    # output lands at partition base 64). Host adds the halves.
    sum_out = nc.dram_tensor("sum_out", [P, D], f32, kind="ExternalOutput")
    sq_out = nc.dram_tensor("sq_out", [P, D], f32, kind="ExternalOutput")

    with tile.TileContext(nc) as tc:
        with (
            tc.tile_pool(name="const", bufs=1) as const_pool,
            # xf bufs == the 8 SW-DMA lanes: slot i always reloaded from the
            # same lane, so the cross-iteration WAW fence is implicit in lane
            # FIFO order and the DMACopy keeps a single sync wait (its ISA
            # struct has one wait slot).
            tc.tile_pool(name="xf", bufs=6) as xf_pool,
            tc.tile_pool(name="xb", bufs=5) as xb_pool,
            tc.tile_pool(name="xq", bufs=5) as xq_pool,
            tc.tile_pool(name="oh", bufs=1) as oh_pool,
            tc.tile_pool(name="res", bufs=1) as res_pool,
            tc.tile_pool(name="psum", bufs=1, space="PSUM") as psum_pool,
        ):
            consts_sb = const_pool.tile([P, n_tiles + batch * N_BINS], f32)
            nc.sync.dma_start(consts_sb[:], consts[:])
            lab_sb = consts_sb[:, :n_tiles]
            # [P, batch, N_BINS] view of the replicated bin indices
            binidx_sb = consts_sb[:, n_tiles:].rearrange(
                "p (j b) -> p j b", b=N_BINS
            )

            # Four accumulation chains, one PSUM bank each: {sum, sumsq} x
            # {even tiles -> array col-groups 0-1 / PSUM partitions 0:50,
            #  odd tiles  -> array col-groups 2-3 / PSUM partitions 64:114}.
            # M=50 uses only 50 of the PE's 128 weight columns, so the even
            # and odd matmuls run CONCURRENTLY in different column groups —
            # halving effective PE time. Separate banks keep each chain's
            # start= bank-clear away from the others.
            acc_s_e = psum_pool.tile([P, D], f32)
            acc_s_o = psum_pool.tile([P, D], f32)
            acc_q_e = psum_pool.tile([P, D], f32)
            acc_q_o = psum_pool.tile([P, D], f32)

            # One persistent buffer holding every tile's one-hot matrix.
            # Unique [:, t, :] slices (never reused) keep the DVE
            # TensorScalar ops at <=1 semaphore wait — walrus rejects more
            # on that opcode — and avoid DVE<->PE slot-recycle stalls.
            oh_all = oh_pool.tile([P, n_tiles, N_BINS], bf16)

            for c in range(n_chunks):
                r0 = c * tiles_per_chunk * P
                chunk = feat[r0 : r0 + tiles_per_chunk * P, :].rearrange(
                    "(j p) d -> p j d", p=P
                )
                # One 2 MiB SWDGE (gpsimd) DMA per chunk halves the per-DMA
                # fixed cost (descriptor emission + completion receipt) that
                # was stretching the chunk cadence past the pure data time.
                # The first and last chunks load in 1 MiB halves instead:
                # the first cast then starts one half-DMA earlier, and the
                # final compute chain begins before the last chunk's second
                # half lands — trimming both the startup and the drain.
                xf = xf_pool.tile([P, tiles_per_chunk, D], f32)
                if c == 0 or c == n_chunks - 1:
                    for hh in range(tiles_per_chunk // batch):
                        s0 = r0 + hh * batch * P
                        sub = feat[s0 : s0 + batch * P, :].rearrange(
                            "(j p) d -> p j d", p=P
                        )
                        nc.gpsimd.dma_start(
                            xf[:, hh * batch : (hh + 1) * batch, :], sub
                        )
                else:
                    nc.gpsimd.dma_start(xf[:], chunk)
                # Compute stays at `batch`-tile granularity so pipeline
                # latency (and the end-of-stream drain) stays short.
                for h in range(tiles_per_chunk // batch):
                    hs = h * batch
                    t0 = c * tiles_per_chunk + hs
                    xfh = xf[:, hs : hs + batch, :]
                    # Casts all on VectorE (ScalarE's activate-copy path is
                    # ~2x slower); squares split 3:1 ScalarE:VectorE so
                    # neither engine approaches the DMA period.
                    nsc = batch - 1
                    xb = xb_pool.tile([P, batch, D], bf16)
                    nc.vector.tensor_copy(xb[:], xfh)
                    xq = xq_pool.tile([P, batch, D], bf16)
                    nc.scalar.activation(
                        xq[:, 0:nsc, :], xb[:, 0:nsc, :],
                        mybir.ActivationFunctionType.Square,
                    )
                    nc.vector.tensor_tensor(
                        xq[:, nsc, :], xb[:, nsc, :], xb[:, nsc, :],
                        op=mybir.AluOpType.mult,
                    )
                    # Whole batch's one-hot matrices in a single DVE op: the
                    # label column broadcasts (stride-0) across the bin axis
                    # against the replicated bin-index block.
                    nc.vector.tensor_tensor(
                        oh_all[:, t0 : t0 + batch, :],
                        binidx_sb,
                        lab_sb[:, t0 : t0 + batch].broadcast_to(
                            [P, batch, N_BINS]
                        ),
                        op=mybir.AluOpType.is_equal,
                    )
                    for j in range(0, batch, 2):
                        t = t0 + j
                        oh_e = oh_all[:, t, :]
                        oh_o = oh_all[:, t + 1, :]
                        first, last = t == 0, t == n_tiles - 2
                        # Interleave even/odd so consecutive matmuls target
                        # different column groups and overlap in the array.
                        nc.tensor.matmul(
                            acc_s_e[0:N_BINS, :], oh_e, xb[:, j, :],
                            start=first, stop=last,
                        )
                        nc.tensor.matmul(
                            acc_s_o[64 : 64 + N_BINS, :], oh_o,
                            xb[:, j + 1, :],
                            start=first, stop=last,
                        )
                        nc.tensor.matmul(
                            acc_q_e[0:N_BINS, :], oh_e, xq[:, j, :],
                            start=first, stop=last,
                        )
                        nc.tensor.matmul(
                            acc_q_o[64 : 64 + N_BINS, :], oh_o,
                            xq[:, j + 1, :],
                            start=first, stop=last,
                        )

            res_s = res_pool.tile([P, D], f32)
            res_q = res_pool.tile([P, D], f32)
            nc.gpsimd.memset(res_s[:], 0.0)
            nc.gpsimd.memset(res_q[:], 0.0)
            nc.vector.tensor_copy(res_s[0:N_BINS, :], acc_s_e[0:N_BINS, :])
            nc.vector.tensor_copy(
                res_s[64 : 64 + N_BINS, :], acc_s_o[64 : 64 + N_BINS, :]
            )
            nc.gpsimd.dma_start(sum_out[:], res_s[:])
            nc.scalar.copy(res_q[0:N_BINS, :], acc_q_e[0:N_BINS, :])
            nc.scalar.copy(
                res_q[64 : 64 + N_BINS, :], acc_q_o[64 : 64 + N_BINS, :]
            )
            # HWDGE path so the two output stores overlap.
            nc.sync.dma_start(sq_out[:], res_q[:])

    nc.finalize()
    return nc


def make_in_maps(features: np.ndarray, labels_f32: np.ndarray) -> list[dict]:
    """Shard host inputs into the 8 per-core input maps."""
    binidx = np.broadcast_to(
        np.arange(N_BINS, dtype=np.float32), (P, BATCH, N_BINS)
    ).reshape(P, BATCH * N_BINS)
    in_maps = []
    for c in range(N_CORES):
        lo = c * ROWS_PER_CORE
        shard = features[lo : lo + ROWS_PER_CORE]
        lab_pt = labels_f32[lo : lo + ROWS_PER_CORE].reshape(N_TILES, P).T
        consts = np.ascontiguousarray(np.concatenate([lab_pt, binidx], axis=1))
        in_maps.append({"feat": shard, "consts": consts})
    return in_maps


def _smooth(x: np.ndarray, w: np.ndarray) -> np.ndarray:
    # torch-style 'reflect' pad (edge-excluding) along bins, then cross-correlate.
    top = x[HALF_KS:0:-1]
    bot = x[-2 : -2 - HALF_KS : -1]
    xp = np.concatenate([top, x, bot], axis=0)
    out = np.zeros_like(x)
    b = x.shape[0]
    for k in range(KS):
        out += w[k] * xp[k : k + b]
    return out


def finalize(
    sums: np.ndarray,
    sumsq: np.ndarray,
    counts: np.ndarray,
    running_mean: np.ndarray,
    running_var: np.ndarray,
    num_samples_tracked: np.ndarray,
    kernel_window: np.ndarray,
):
    """Replicates reference() from the per-bin partials (float64 internally)."""
    counts = counts.astype(np.float64)
    safe_n = np.maximum(counts, 1.0)[:, None]
    mean = sums / safe_n
    var = (sumsq - safe_n * mean * mean) / np.maximum(counts - 1.0, 1.0)[:, None]

    present = (counts > 0.0)[:, None]
    f = MOMENTUM
    rm = running_mean.astype(np.float64)
    rv = running_var.astype(np.float64)
    new_mean = np.where(present, (1.0 - f) * mean + f * rm, rm)
    new_var = np.where(present, (1.0 - f) * var + f * rv, rv)
    new_num = num_samples_tracked.astype(np.float64) + counts

    w = kernel_window.astype(np.float64)
    smoothed_mean = _smooth(new_mean, w)
    smoothed_var = _smooth(new_var, w)
    return (
        new_mean.astype(np.float32),
        new_var.astype(np.float32),
        new_num.astype(np.float32),
        smoothed_mean.astype(np.float32),
        smoothed_var.astype(np.float32),
    )


def kernel(features, labels, running_mean, running_var, num_samples_tracked, kernel_window):
    features = np.ascontiguousarray(np.asarray(features), dtype=np.float32)
    labels_i = np.asarray(labels).astype(np.int64)
    labels_f32 = labels_i.astype(np.float32)
    running_mean = np.asarray(running_mean, dtype=np.float32)
    running_var = np.asarray(running_var, dtype=np.float32)
    num_samples_tracked = np.asarray(num_samples_tracked, dtype=np.float32)
    kernel_window = np.asarray(kernel_window, dtype=np.float32)

    if "nc" not in _NC_CACHE:
        _NC_CACHE["nc"] = build_nc()
    nc = _NC_CACHE["nc"]

    in_maps = make_in_maps(features, labels_f32)
    res = run_bass_kernel_spmd(nc, in_maps, list(range(N_CORES))).results

    sums = np.sum(
        [r["sum_out"][0:N_BINS] + r["sum_out"][64 : 64 + N_BINS] for r in res],
        axis=0, dtype=np.float64,
    )
    sumsq = np.sum(
        [r["sq_out"][0:N_BINS] + r["sq_out"][64 : 64 + N_BINS] for r in res],
        axis=0, dtype=np.float64,
    )
    counts = np.bincount(labels_i, minlength=N_BINS).astype(np.float64)

    return finalize(
        sums, sumsq, counts,
        running_mean, running_var, num_samples_tracked, kernel_window,
    )
